# revision 5
# baseline (speedup 1.0000x reference)
"""4-D average pool (kernel=2, stride=2) over [2,16,32,32,32,32] f32, on 8 NeuronCores.

Strategy: data-parallel over the 32 (b,c) slices -> 4 slices per core; the
per-core input is a contiguous [4096, 1024] f32 block (rows = (slice,d1,d2),
cols = (d3,d4)).

The partitioning puts ALL four pooled dims in the free dimension, so the
whole reduction runs on DVE adds -- no PE matmul, no PSUM, no ACTIVATE:
  chunk j = 512 consecutive rows (slice = j>>1, d1 bit4 = j&1) -- a
  contiguous 2 MiB DRAM window; within it
  partition p = d1m*16 + d2m   (d1m = d1 bits 3:1, d2m = d2 bits 4:1)
  free = (d1l 2, d2l 2, d3 32, d4 32)
Each chunk loads as 8 window DMAs of 64 CONTIGUOUS rows (256 KiB) into 16
partitions each -- per-engine DMA address streams stay sequential, which is
what the HBM needs for full rate (scattered 8 KiB gathers measure ~2x
slower under 8-core load).  Windows alternate between the SP and ACT HWDGE
rings (d1m 0-3 / 4-7 -> even/odd SDMA engine sets) so both engine halves
stay fed; the rings carry nothing else, and the whole 16 MiB shard is
SBUF-resident so no load ever waits.

Per chunk DVE pools d4, d3, d2, d1 pairs (4 halving adds) plus a
tensor_scalar for the 1/16; outputs land in y[128j : 128j+128] so each
store is one full-width contiguous 128 KiB DMA on the GPSIMD SWDGE path
(its own descriptor queue -- HWDGE ring FIFOs would defer store data
behind all remaining load descriptors).  DVE runs ~50% occupied, so the
gapless load stream stays the critical path.  The last chunk is split by
d1l into two half-window sets so only a ~1.5 us DVE chain + one store
trail the final 128 KiB load, and the ~120-instruction kernel keeps the
iram-load preamble and event-semaphore teardown short.
"""

import sys

import numpy as np

if "/opt/trn_rl_repo" not in sys.path:
    sys.path.insert(0, "/opt/trn_rl_repo")

import concourse.bacc as bacc
import concourse.bass as bass
import concourse.tile as tile
from concourse import mybir
from concourse.bass_utils import run_bass_kernel_spmd

N_CORES = 8
SLICES_PER_CORE = 4  # 32 (b,c) slices / 8 cores
ROWS = SLICES_PER_CORE * 1024  # 4096
F32 = mybir.dt.float32


def build_nc() -> bass.Bass:
    # Bacc (not raw Bass): its compile() splits multi-sem sync waits into
    # event-semaphore instructions (TRN2 allows one wait per instruction).
    nc = bacc.Bacc()
    x = nc.dram_tensor("x", [ROWS, 1024], F32, kind="ExternalInput")
    y = nc.dram_tensor("y", [ROWS // 4, 256], F32, kind="ExternalOutput")

    with tile.TileContext(nc) as tc:
        with (
            # bufs = chunk count -> every chunk gets its own slot; the whole
            # 16 MiB shard is SBUF-resident so load DMAs never wait
            tc.tile_pool(name="inp", bufs=7) as inp,
            tc.tile_pool(name="inh", bufs=2) as inh,
            tc.tile_pool(name="m1p", bufs=2) as m1p,
            tc.tile_pool(name="m2p", bufs=2) as m2p,
            tc.tile_pool(name="m3p", bufs=4) as m3p,
            tc.tile_pool(name="m4p", bufs=2) as m4p,
            tc.tile_pool(name="obp", bufs=4) as obp,
        ):

            def ring(k):
                # window k covers partitions [16k, 16k+16): d1m 0-3 hit the
                # even SDMA engine set, 4-7 the odd -- one HWDGE ring each
                return nc.sync if k < 4 else nc.scalar

            def pool_d4_d3(tv, nd1l, mtag):
                # tv free = (d1l nd1l, d2l 2, d3 32, d4 32)
                a = nd1l * 2 * 32
                v = tv.rearrange("p (a o4 e4) -> p a o4 e4", a=a, o4=16)
                m1 = m1p.tile([128, a * 16], F32, tag=f"{mtag}1")
                m1v = m1[:].rearrange("p (a o4) -> p a o4", a=a)
                nc.vector.tensor_add(m1v, v[:, :, :, 0], v[:, :, :, 1])
                b = nd1l * 2
                w = m1[:].rearrange(
                    "p (b o3 e3 o4) -> p b o3 e3 o4", b=b, o3=16, e3=2
                )
                m2 = m2p.tile([128, b * 256], F32, tag=f"{mtag}2")
                m2v = m2[:].rearrange("p (b o3 o4) -> p b o3 o4", b=b, o3=16)
                nc.vector.tensor_add(m2v, w[:, :, :, 0, :], w[:, :, :, 1, :])
                return m2

            def finish_and_store(m4, j):
                ob = obp.tile([128, 256], F32, tag="ob")
                nc.vector.tensor_scalar_mul(ob[:], m4[:], 1.0 / 16.0)
                nc.gpsimd.dma_start(y[128 * j : 128 * j + 128, :], ob[:])

            for j in range(7):  # full 2 MiB chunks
                t = inp.tile([128, 4096], F32, tag="t")
                for k in range(8):
                    base = 512 * j + 64 * k
                    # window rows r = d1l*32 + d2m*2 + d2l -> partition d2m
                    src = x[base : base + 64, :].rearrange(
                        "(d1l d2m d2l) c -> d2m d1l (d2l c)", d1l=2, d2m=16
                    )
                    dst = t[:][16 * k : 16 * k + 16, :].rearrange(
                        "q (d1l c) -> q d1l c", d1l=2
                    )
                    ring(k).dma_start(dst, src)
                m2 = pool_d4_d3(t[:], 2, "f")
                # pool d2l pairs: free (d1l 2, e2 2, c 256)
                z = m2[:].rearrange("p (d1l e2 c) -> p d1l e2 c", d1l=2, e2=2)
                m3 = m3p.tile([128, 512], F32, tag="m3f")
                m3v = m3[:].rearrange("p (d1l c) -> p d1l c", d1l=2)
                nc.vector.tensor_add(m3v, z[:, :, 0, :], z[:, :, 1, :])
                # pool the d1l pair
                zz = m3[:].rearrange("p (d1l c) -> p d1l c", d1l=2)
                m4 = m4p.tile([128, 256], F32, tag="m4")
                nc.vector.tensor_add(m4[:], zz[:, 0, :], zz[:, 1, :])
                finish_and_store(m4, j)

            # last chunk split by d1l (half-windows of 32 contiguous rows):
            # only a short DVE chain + one store trail the final load
            m3s = []
            for g in range(2):
                t = inh.tile([128, 2048], F32, tag="th")
                for k in range(8):
                    base = 512 * 7 + 64 * k + 32 * g
                    src = x[base : base + 32, :].rearrange(
                        "(d2m d2l) c -> d2m (d2l c)", d2m=16
                    )
                    ring(k).dma_start(t[:][16 * k : 16 * k + 16, :], src)
                m2 = pool_d4_d3(t[:], 1, "h")
                z = m2[:].rearrange("p (e2 c) -> p e2 c", e2=2)
                m3 = m3p.tile([128, 256], F32, tag="m3h")
                nc.vector.tensor_add(m3[:], z[:, 0, :], z[:, 1, :])
                m3s.append(m3)
            m4 = m4p.tile([128, 256], F32, tag="m4")
            nc.vector.tensor_add(m4[:], m3s[0][:], m3s[1][:])
            finish_and_store(m4, 7)

    nc.compile()
    return nc


_NC_CACHE: bass.Bass | None = None


def kernel(nd_tensor: np.ndarray, _trace: bool = False):
    global _NC_CACHE
    x = np.ascontiguousarray(np.asarray(nd_tensor, dtype=np.float32)).reshape(
        32, 1024, 1024
    )
    if _NC_CACHE is None:
        _NC_CACHE = build_nc()
    nc = _NC_CACHE

    in_maps = [
        {
            "x": np.ascontiguousarray(
                x[SLICES_PER_CORE * i : SLICES_PER_CORE * (i + 1)]
            ).reshape(ROWS, 1024),
        }
        for i in range(N_CORES)
    ]
    res = run_bass_kernel_spmd(
        nc, in_maps, core_ids=list(range(N_CORES)), trace=_trace
    )
    out = np.stack([res.results[i]["y"] for i in range(N_CORES)])  # [8,1024,256]
    out = out.reshape(2, 16, 16, 16, 16, 16).astype(np.float32)
    if _trace:
        kernel.last_results = res
    return out


# revision 7
# speedup vs baseline: 2.6143x; 2.6143x over previous
"""4-D average pool (kernel=2, stride=2) over [2,16,32,32,32,32] f32, on 8 NeuronCores.

Strategy: data-parallel over the 32 (b,c) slices -> 4 slices per core; the
per-core input is a contiguous [4096, 1024] f32 block (rows = (slice,d1,d2),
cols = (d3,d4)).

DMA ground rules (measured): only big CONTIGUOUS loads on a single HWDGE
ring sustain the ~383 GB/s per-core HBM rate under 8-core load; scattered
8 KiB gathers run ~2x slower and small (<=512 KiB) DMAs serialize on the
ring at ~2 us each.  So loads are contiguous [256q]-row blocks on the SP
ring with partition p = row-pair index (p = (row>>1) & 127, 8 KiB
descriptors).  That puts d2's low bit in the FREE dim -- DVE pools the d4,
d3, d2 pairs with three halving adds -- and leaves only d1's low bit in the
partition dim (p bit 4).  One bf16 matmul with a constant [128, 64] pairing
matrix (1/16 scale folded in, exact in bf16) pools it at ~1 ns/col -- 4x
cheaper than the f32 [128->32] alternative -- then ScalarE copies PSUM ->
SBUF f32 and the ACT-ring store writes y[64u : 64u+64q] contiguously.

The whole 16 MiB shard stays SBUF-resident so no load ever waits; DVE runs
~45% occupied and PE ~10%, so the gapless 43.8 us load stream is the
critical path.  The loads taper to two 1 MiB (256-row) chunks so only a
~3 us add/matmul/store chain trails the final byte (vs ~10 us for the f32
matmul pipeline), and the ~80-instruction kernel keeps the iram-load
preamble and event-semaphore teardown short.
"""

import sys

import numpy as np

if "/opt/trn_rl_repo" not in sys.path:
    sys.path.insert(0, "/opt/trn_rl_repo")

import concourse.bacc as bacc
import concourse.bass as bass
import concourse.tile as tile
from concourse import mybir
from concourse.bass_utils import run_bass_kernel_spmd

N_CORES = 8
SLICES_PER_CORE = 4  # 32 (b,c) slices / 8 cores
ROWS = SLICES_PER_CORE * 1024  # 4096
F32 = mybir.dt.float32
BF16 = mybir.dt.bfloat16
# chunk sizes in 256-row (1 MiB) units: 2 MiB bulk, 1 MiB tail taper
UNITS = [2, 2, 2, 2, 2, 2, 2, 1, 1]


def _build_pm() -> np.ndarray:
    # pm[p, o] = 1/16 iff o = (p>>5)*16 + (p&15): pools partition bit 4
    # (= d1 low bit) and applies the average scale (0.0625 is exact in bf16)
    import ml_dtypes

    pm = np.zeros((128, 64), np.float32)
    for p in range(128):
        pm[p, (p >> 5) * 16 + (p & 15)] = 1.0 / 16.0
    return pm.astype(ml_dtypes.bfloat16)


def build_nc() -> bass.Bass:
    # Bacc (not raw Bass): its compile() splits multi-sem sync waits into
    # event-semaphore instructions (TRN2 allows one wait per instruction).
    nc = bacc.Bacc()
    x = nc.dram_tensor("x", [ROWS, 1024], F32, kind="ExternalInput")
    pm = nc.dram_tensor("pm", [128, 64], BF16, kind="ExternalInput")
    y = nc.dram_tensor("y", [ROWS // 4, 256], F32, kind="ExternalOutput")

    with tile.TileContext(nc) as tc:
        with (
            tc.tile_pool(name="pmp", bufs=1) as pmp,
            # one slot per chunk -> the whole 16 MiB shard is SBUF-resident,
            # so load DMAs carry no waits and stream back-to-back
            tc.tile_pool(name="inp", bufs=len(UNITS)) as inp,
            tc.tile_pool(name="m1p", bufs=2) as m1p,
            tc.tile_pool(name="m2p", bufs=2) as m2p,
            tc.tile_pool(name="m3p", bufs=4) as m3p,
            tc.tile_pool(name="psp", bufs=4, space=bass.MemorySpace.PSUM) as psp,
            tc.tile_pool(name="obp", bufs=4) as obp,
        ):
            pm_t = pmp.tile([128, 64], BF16)

            B = 0
            for ci, q in enumerate(UNITS):
                # contiguous [256q, 1024] block; partition = row-pair index
                t = inp.tile([128, 2048 * q], F32, tag="t")
                src = x[256 * B : 256 * (B + q), :].rearrange(
                    "(q p r0) c -> p q (r0 c)", p=128, r0=2
                )
                nc.sync.dma_start(
                    t[:].rearrange("p (q c) -> p q c", q=q), src
                )
                if ci == 0:
                    # pm load after the first bulk DMA: only needed by the
                    # first matmul (~7 us in), off the critical path
                    nc.sync.dma_start(pm_t[:], pm[:])

                # free = (q, d2l 2, d3 32, d4 32)
                a = q * 64
                v = t[:].rearrange("p (a o4 e4) -> p a o4 e4", a=a, o4=16)
                m1 = m1p.tile([128, a * 16], F32, tag="m1")
                m1v = m1[:].rearrange("p (a o4) -> p a o4", a=a)
                nc.vector.tensor_add(m1v, v[:, :, :, 0], v[:, :, :, 1])

                b = q * 2
                w = m1[:].rearrange(
                    "p (b o3 e3 o4) -> p b o3 e3 o4", b=b, o3=16, e3=2
                )
                m2 = m2p.tile([128, b * 256], F32, tag="m2")
                m2v = m2[:].rearrange("p (b o3 o4) -> p b o3 o4", b=b, o3=16)
                nc.vector.tensor_add(m2v, w[:, :, :, 0, :], w[:, :, :, 1, :])

                # pool d2l pairs, casting to bf16 for the cheap matmul
                z = m2[:].rearrange("p (qq e2 c) -> p qq e2 c", qq=q, e2=2)
                m3 = m3p.tile([128, q * 256], BF16, tag="m3")
                m3v = m3[:].rearrange("p (qq c) -> p qq c", qq=q)
                nc.vector.tensor_add(m3v, z[:, :, 0, :], z[:, :, 1, :])

                # pool the d1l partition pairs (+1/16 scale) in one matmul
                ps = psp.tile([64, q * 256], F32, tag="ps")
                nc.tensor.matmul(ps[:], pm_t[:], m3[:], start=True, stop=True)
                ob = obp.tile([64, q * 256], F32, tag="ob")
                nc.scalar.copy(ob[:], ps[:])

                # unit u's 64 output rows are y[64u : 64u+64] -- contiguous
                dst = y[64 * B : 64 * (B + q), :].rearrange(
                    "(q r) c -> r q c", r=64
                )
                nc.scalar.dma_start(dst, ob[:].rearrange("r (q c) -> r q c", q=q))
                B += q

    nc.compile()
    return nc


_NC_CACHE: bass.Bass | None = None


def kernel(nd_tensor: np.ndarray, _trace: bool = False):
    global _NC_CACHE
    x = np.ascontiguousarray(np.asarray(nd_tensor, dtype=np.float32)).reshape(
        32, 1024, 1024
    )
    if _NC_CACHE is None:
        _NC_CACHE = build_nc()
    nc = _NC_CACHE
    pm = _build_pm()

    in_maps = [
        {
            "x": np.ascontiguousarray(
                x[SLICES_PER_CORE * i : SLICES_PER_CORE * (i + 1)]
            ).reshape(ROWS, 1024),
            "pm": pm,
        }
        for i in range(N_CORES)
    ]
    res = run_bass_kernel_spmd(
        nc, in_maps, core_ids=list(range(N_CORES)), trace=_trace
    )
    out = np.stack([res.results[i]["y"] for i in range(N_CORES)])  # [8,1024,256]
    out = out.reshape(2, 16, 16, 16, 16, 16).astype(np.float32)
    if _trace:
        kernel.last_results = res
    return out
